# revision 46
# baseline (speedup 1.0000x reference)
# Trainium2 Bass/Tile kernel for causal GQA attention (dense_transformer).
#
# Reference computation (fp32):
#   Q = x@wq, K = x@wk, V = x@wv  (rotary on Q,K; GQA 32 q heads / 8 kv heads)
#   out = softmax(QK^T/sqrt(64), causal) @ V @ wo
#
# Sharding: tensor-parallel over heads (TP=4: 8 q heads + 2 kv heads per
# core) x data-parallel over batch (DP=2: 2 batches per core) = 8 cores.
# Each core computes a partial [2,1024,2048] output (its heads' wo
# contribution); host sums partials within each DP group.
#
# Device pipeline per core (all matmuls bf16 -> fp32 PSUM), software
# pipeline with batch-interleaved tile order:
#   av(i-1):    transposed AV: per (kv group, k-chunk) two F=256 matmuls
#     with stationary [V|ones] -> av^T psum [128, 512] whose rows 64:128
#     are 64 copies of the softmax denominator; ACT copy + fast-approx
#     reciprocal + one DVE multiply normalizes straight into the WO
#     stationary layout (no attn transposes, no per-head reciprocals).
#     Issued BEFORE phase1(i) so the normalize chain hides under proj.
#   phase1(i):  Q and fused K|V projections (contract D=2048 from
#     host-pretransposed x^T tiles), rotary on Q/K via DVE in natural
#     layout, PE-transpose Q/K to head-major [d, q] layout, V written
#     to a bf16 arena [k, chunk, 0:64=V | 64:128=ones].
#   wo(i-1):    W_O, 512-wide psum chunks, evac on ACT/DVE, DMA out.
#   scores(i):  scoresT = K^T-chunk.T @ Q^T, two heads of a pair as
#     concurrent row-tiled matmuls into separate PSUM banks; exp on ACT
#     (no max subtraction - scores are bounded ~6), diagonal causal mask
#     via gpsimd multiply; P^T parked in SBUF.
import numpy as np
import ml_dtypes

B, S, D = 4, 1024, 2048
NH, NKV, HD = 32, 8, 64
TP, DP = 4, 2
QH = NH // TP            # 8 q heads per core
KVH = NKV // TP          # 2 kv heads per core
BL = B // DP             # 2 batches per core
NT = S // 128            # 8 s-tiles per batch
NQT = BL * NT            # 16 q-tiles per core
DC = D // 128            # 16 contraction chunks for the projections
PAIRS = QH // 2          # 4 head pairs (h, h+4) packed per 128 partitions
SCALE = 1.0 / float(np.sqrt(HD))
PERM = [0, 4, 1, 5, 2, 6, 3, 7]   # local head order: pair p = (p, p+4)

bf = ml_dtypes.bfloat16

_built = None


def _build():
    from contextlib import ExitStack
    import concourse.bacc as bacc
    import concourse.tile as tile
    from concourse import mybir

    f32 = mybir.dt.float32
    b16 = mybir.dt.bfloat16
    Exp = mybir.ActivationFunctionType.Exp

    nc = bacc.Bacc("TRN2", target_bir_lowering=False, debug=False,
                   num_devices=TP * DP)

    xt_d = nc.dram_tensor("xt", [NQT, 128, DC, 128], b16, kind="ExternalInput").ap()
    # weights grouped 4 contraction-chunks per DMA: [group, 128, 4, N]
    wq_d = nc.dram_tensor("wqr", [DC // 4, 128, 4, QH * HD], b16, kind="ExternalInput").ap()
    wkv_d = nc.dram_tensor("wkvr", [DC // 4, 128, 4, 2 * KVH * HD], b16, kind="ExternalInput").ap()
    wo_d = nc.dram_tensor("wor", [128, PAIRS, D], b16, kind="ExternalInput").ap()
    cos_d = nc.dram_tensor("cosr", [128, NT, QH * HD // 2], b16, kind="ExternalInput").ap()
    sin_d = nc.dram_tensor("sinr", [128, NT, QH * HD // 2], b16, kind="ExternalInput").ap()
    mask_d = nc.dram_tensor("maskr", [128, 512], b16, kind="ExternalInput").ap()
    id_d = nc.dram_tensor("identr", [128, 128], b16, kind="ExternalInput").ap()
    y_d = nc.dram_tensor("y", [NQT, 128, D], b16, kind="ExternalOutput").ap()

    with tile.TileContext(nc) as tc:
        with ExitStack() as ctx:
            singles = ctx.enter_context(tc.tile_pool(name="singles", bufs=1))
            # PSUM: 8 banks: 3 proj + 2 scores/tq + 3 av/wo
            pp = ctx.enter_context(tc.tile_pool(name="pp", bufs=2, space="PSUM"))
            psc = ctx.enter_context(tc.tile_pool(name="psc", bufs=4, space="PSUM"))
            pav = ctx.enter_context(tc.tile_pool(name="pav", bufs=2, space="PSUM"))
            xtp = ctx.enter_context(tc.tile_pool(name="xtp", bufs=5))
            rot = ctx.enter_context(tc.tile_pool(name="rot", bufs=2))
            rtmp = ctx.enter_context(tc.tile_pool(name="rtmp", bufs=4))
            persist = ctx.enter_context(tc.tile_pool(name="persist", bufs=1))
            ptp = ctx.enter_context(tc.tile_pool(name="ptp", bufs=1))
            anp = ctx.enter_context(tc.tile_pool(name="anp", bufs=3))
            outp = ctx.enter_context(tc.tile_pool(name="outp", bufs=3))

            mask_sb = singles.tile([128, 512], b16)
            ident_sb = singles.tile([128, 128], b16)
            wq_g = [singles.tile([128, 4, QH * HD], b16, name=f"wq{g}")
                    for g in range(DC // 4)]
            wkv_g = [singles.tile([128, 4, 2 * KVH * HD], b16, name=f"wkv{g}")
                     for g in range(DC // 4)]
            cos_sb = singles.tile([128, NT, QH * HD // 2], b16)
            sin_sb = singles.tile([128, NT, QH * HD // 2], b16)
            wo_sb = singles.tile([128, PAIRS, D], b16)
            # V arena per (local batch, kv head): [k, chunk, 0:64=V|64:128=1]
            varena = [[singles.tile([128, NT, 128], b16, name=f"va{b}_{kv}")
                       for kv in range(KVH)] for b in range(BL)]
            for b_ in range(BL):
                for kv in range(KVH):
                    nc.vector.memset(varena[b_][kv][:, :, HD:128], 1.0)

            xt_sbs = {}

            def load_xt(i, eng):
                xt_sb = xtp.tile([128, DC, 128], b16, tag="xt", name=f"xt{i}")
                eng.dma_start(out=xt_sb, in_=xt_d[i])
                xt_sbs[i] = xt_sb

            def load_xt_split(i, eng1, eng2, cut=4):
                xt_sb = xtp.tile([128, DC, 128], b16, tag="xt", name=f"xt{i}")
                eng1.dma_start(out=xt_sb[:, 0:cut, :], in_=xt_d[i][:, 0:cut, :])
                eng2.dma_start(out=xt_sb[:, cut:DC, :], in_=xt_d[i][:, cut:DC, :])
                xt_sbs[i] = xt_sb

            # Startup-critical bytes split finely across the DGE queues so
            # each queue's FIRST transfer is exactly what the PE needs first.
            nc.sync.dma_start(out=wq_g[0][:, 0, :], in_=wq_d[0][:, 0, :])
            nc.gpsimd.dma_start(out=wkv_g[0][:, 0:2, :], in_=wkv_d[0][:, 0:2, :])
            load_xt_split(0, nc.scalar, nc.scalar)
            nc.sync.dma_start(out=wq_g[0][:, 1:4, :], in_=wq_d[0][:, 1:4, :])
            nc.gpsimd.dma_start(out=wkv_g[0][:, 2:4, :], in_=wkv_d[0][:, 2:4, :])
            nc.gpsimd.dma_start(out=cos_sb[:, 0:2, :], in_=cos_d[:, 0:2, :])
            nc.gpsimd.dma_start(out=sin_sb[:, 0:2, :], in_=sin_d[:, 0:2, :])
            nc.sync.dma_start(out=wq_g[1], in_=wq_d[1])
            nc.gpsimd.dma_start(out=ident_sb, in_=id_d)
            nc.scalar.dma_start(out=wkv_g[1], in_=wkv_d[1])
            load_xt(NT, nc.sync)
            nc.gpsimd.dma_start(out=mask_sb, in_=mask_d)
            nc.sync.dma_start(out=wq_g[2], in_=wq_d[2])
            nc.gpsimd.dma_start(out=wkv_g[2], in_=wkv_d[2])
            load_xt(1, nc.scalar)
            nc.sync.dma_start(out=wq_g[3], in_=wq_d[3])
            nc.scalar.dma_start(out=wkv_g[3], in_=wkv_d[3])
            nc.sync.dma_start(out=wo_sb, in_=wo_d)

            qt_tiles = {}
            kt_tiles = {}
            pt_tiles = {}

            def phase1(i, idx):
                bl, t = divmod(i, NT)
                # ---------- QKV projection ----------
                if idx == 0 and idx + 4 < len(order):
                    # halves on different queues: the consumer reads chunks
                    # sequentially, so the late half has ~2.6us of slack
                    load_xt_split(order[idx + 3], nc.gpsimd, nc.sync, cut=8)
                    load_xt_split(order[idx + 4], nc.sync, nc.gpsimd, cut=8)
                elif idx + 4 < len(order):
                    load_xt(order[idx + 4],
                            nc.sync if idx % 2 == 0 else nc.gpsimd)
                if idx == 1:
                    nc.gpsimd.dma_start(out=cos_sb[:, 2:4, :], in_=cos_d[:, 2:4, :])
                    nc.gpsimd.dma_start(out=sin_sb[:, 2:4, :], in_=sin_d[:, 2:4, :])
                if idx == 4:
                    nc.gpsimd.dma_start(out=cos_sb[:, 4:6, :], in_=cos_d[:, 4:6, :])
                    nc.gpsimd.dma_start(out=sin_sb[:, 4:6, :], in_=sin_d[:, 4:6, :])
                if idx == 8:
                    nc.gpsimd.dma_start(out=cos_sb[:, 6:NT, :], in_=cos_d[:, 6:NT, :])
                    nc.gpsimd.dma_start(out=sin_sb[:, 6:NT, :], in_=sin_d[:, 6:NT, :])
                xt_sb = xt_sbs.pop(i)
                q_ps = pp.tile([128, QH * HD], f32, tag="pp", name=f"qps{i}")
                kv_ps = pp.tile([128, 2 * KVH * HD], f32, tag="pp", name=f"kvps{i}")
                for c in range(DC):
                    st, sp = (c == 0), (c == DC - 1)
                    g, j = divmod(c, 4)
                    nc.tensor.matmul(q_ps, xt_sb[:, c, :], wq_g[g][:, j, :],
                                     start=st, stop=sp)
                    nc.tensor.matmul(kv_ps, xt_sb[:, c, :], wkv_g[g][:, j, :],
                                     start=st, stop=sp)

                # ---------- rotary (natural layout, pairs on free dim) ----
                c_sl = cos_sb[:, t, :]
                s_sl = sin_sb[:, t, :]
                qrot = rot.tile([128, QH * HD], b16, tag="qrot", name=f"qr{i}")
                qv = qrot.rearrange("p (n two) -> p two n", two=2)
                qp = q_ps.rearrange("p (n two) -> p two n", two=2)
                t1 = rtmp.tile([128, QH * HD // 2], f32, tag="t1", name=f"t1a{i}")
                t2 = rtmp.tile([128, QH * HD // 2], f32, tag="t2", name=f"t2a{i}")
                nc.vector.tensor_mul(t1, qp[:, 0, :], c_sl)
                nc.vector.tensor_mul(t2, qp[:, 1, :], s_sl)
                nc.vector.tensor_sub(qv[:, 0, :], t1, t2)
                t3 = rtmp.tile([128, QH * HD // 2], f32, tag="t1", name=f"t1b{i}")
                t4 = rtmp.tile([128, QH * HD // 2], f32, tag="t2", name=f"t2b{i}")
                nc.vector.tensor_mul(t3, qp[:, 0, :], s_sl)
                nc.vector.tensor_mul(t4, qp[:, 1, :], c_sl)
                nc.vector.tensor_add(qv[:, 1, :], t3, t4)

                ck_sl = cos_sb[:, t, 0:KVH * HD // 2]
                sk_sl = sin_sb[:, t, 0:KVH * HD // 2]
                krot = rot.tile([128, KVH * HD], b16, tag="krot", name=f"kr{i}")
                kv_ = krot.rearrange("p (n two) -> p two n", two=2)
                kp = kv_ps[:, 0:KVH * HD].rearrange("p (n two) -> p two n", two=2)
                u1 = rtmp.tile([128, KVH * HD // 2], f32, tag="u1", name=f"u1a{i}")
                u2 = rtmp.tile([128, KVH * HD // 2], f32, tag="u2", name=f"u2a{i}")
                nc.vector.tensor_mul(u1, kp[:, 0, :], ck_sl)
                nc.vector.tensor_mul(u2, kp[:, 1, :], sk_sl)
                nc.vector.tensor_sub(kv_[:, 0, :], u1, u2)
                u3 = rtmp.tile([128, KVH * HD // 2], f32, tag="u1", name=f"u1b{i}")
                u4 = rtmp.tile([128, KVH * HD // 2], f32, tag="u2", name=f"u2b{i}")
                nc.vector.tensor_mul(u3, kp[:, 0, :], sk_sl)
                nc.vector.tensor_mul(u4, kp[:, 1, :], ck_sl)
                nc.vector.tensor_add(kv_[:, 1, :], u3, u4)

                # ---------- V -> bf16 arena (ones cols are static) ----------
                voff = KVH * HD
                for kv in range(KVH):
                    nc.vector.tensor_copy(
                        varena[bl][kv][:, t, 0:HD],
                        kv_ps[:, voff + kv * HD:voff + (kv + 1) * HD])

                # ---------- PE transposes to [d, q]: 4 Q pairs + K into one
                # psum bank, evacuated with a single wide DVE copy ----------
                tq_ps = psc.tile([128, PAIRS + 1, 128], b16, tag="sc",
                                 name=f"tq{i}")
                for j in range(PAIRS):
                    nc.tensor.transpose(tq_ps[:, j, :],
                                        qrot[:, j * 128:(j + 1) * 128], ident_sb)
                nc.tensor.transpose(tq_ps[:, PAIRS, :], krot, ident_sb)
                qkt_sb = persist.tile([128, PAIRS + 1, 128], b16, tag="qt",
                                      bufs=2 * NT + 4, name=f"qkt{i}")
                nc.vector.tensor_copy(qkt_sb, tq_ps)
                qt_tiles[i] = qkt_sb
                kt_tiles[i] = qkt_sb[:, PAIRS, :]

            def score_group(i, pg, g0):
                bl, t = divmod(i, NT)
                ib = bl * NT
                nkc = t + 1
                # two pairs per matmul: moving = [64, 2 pairs, 128 q] (256
                # cols); A/B halves alternate PE row-tiles so ldweights hide
                qkt = qt_tiles[i]
                qtt2A = qkt[0:64, 2 * pg:2 * pg + 2, :].rearrange(
                    "p a b -> p (a b)")
                qtt2B = qkt[64:128, 2 * pg:2 * pg + 2, :].rearrange(
                    "p a b -> p (a b)")
                gk = min(2, nkc - g0)
                scA = psc.tile([128, 512], f32, tag="sc",
                               name=f"scA{i}_{pg}_{g0}")
                scB = psc.tile([128, 512], f32, tag="sc",
                               name=f"scB{i}_{pg}_{g0}")
                for gi in range(gk):
                    ktt = kt_tiles[ib + g0 + gi]
                    o = gi * 256
                    nc.tensor.matmul(scA[:, o:o + 256],
                                     ktt[0:64, :], qtt2A,
                                     start=True, stop=True)
                    nc.tensor.matmul(scB[:, o:o + 256],
                                     ktt[64:128, :], qtt2B,
                                     start=True, stop=True)
                if (i, g0) not in pt_tiles:
                    pt_tiles[(i, g0)] = (
                        ptp.tile([128, 2, 512], b16, tag="pt", bufs=20,
                                 name=f"ptA{i}_{g0}"),
                        ptp.tile([128, 2, 512], b16, tag="pt", bufs=20,
                                 name=f"ptB{i}_{g0}"))
                ptA2, ptB2 = pt_tiles[(i, g0)]
                ptA, ptB = ptA2[:, pg, :], ptB2[:, pg, :]
                nc.scalar.activation(ptA[:, 0:gk * 256], scA[:, 0:gk * 256],
                                     Exp, scale=SCALE)
                nc.scalar.activation(ptB[:, 0:gk * 256], scB[:, 0:gk * 256],
                                     Exp, scale=SCALE)
                if g0 + gk == nkc:   # diagonal chunk is last
                    o = (gk - 1) * 256
                    nc.gpsimd.tensor_mul(ptA[:, o:o + 256],
                                         ptA[:, o:o + 256],
                                         mask_sb[:, 0:256])
                    nc.gpsimd.tensor_mul(ptB[:, o:o + 256],
                                         ptB[:, o:o + 256],
                                         mask_sb[:, 0:256])

            def score_groups_of(i):
                bl, t = divmod(i, NT)
                return [(pg, g0) for pg in range(PAIRS // 2)
                        for g0 in range(0, t + 1, 2)]

            def av_alloc(i):
                return [pav.tile([128, 512], f32, tag="av", name=f"av{i}_{kv}")
                        for kv in range(KVH)]

            def av_pass(i, avps, kv):
                # transposed AV: per (kv, kc) ONE F=512 matmul, stationary
                # [V|ones]; moving = both pg groups' P^T for chunk kc via a
                # 3D strided AP [128, (pg 2, stride 512), 256]
                bl, t = divmod(i, NT)
                nkc = t + 1
                avp = avps[kv]
                for kc in range(nkc):
                    g0, gi = (kc // 2) * 2, kc % 2
                    ptA2, ptB2 = pt_tiles[(i, g0)]
                    pt_ = ptA2 if kv == 0 else ptB2
                    o = gi * 256
                    nc.tensor.matmul(avp, varena[bl][kv][:, kc, :],
                                     pt_[:, :, o:o + 256],
                                     start=(kc == 0), stop=(kc == nkc - 1))

            def av_done(i):
                bl, t = divmod(i, NT)
                for g02 in range(0, t + 1, 2):
                    pt_tiles.pop((i, g02), None)

            def av_normalize(i, avps, aT_sb, kv):
                # rows 64:128 of avp = 64 copies of the softmax denominator
                # (approx-recip is a custom DVE op: needs SBUF input)
                avp = avps[kv]
                denb = anp.tile([64, 512], f32, tag="denb",
                                name=f"db{i}_{kv}")
                nc.scalar.copy(denb, avp[64:128, :])
                recip = anp.tile([64, 512], f32, tag="recip",
                                 name=f"rc{i}_{kv}")
                nc.vector.reciprocal_approx_fast(out=recip, in_=denb)
                nc.vector.tensor_mul(
                    aT_sb[kv * 64:(kv + 1) * 64, :, :].rearrange(
                        "p a b -> p (a b)"),
                    avp[0:64, :], recip)

            def wo_tail(i, aT_sb, halves=(0, 1), out_sb=None):
                if out_sb is None:
                    out_sb = outp.tile([128, D], b16, tag="out", name=f"o{i}")
                # two 512-wide output chunks in flight so each attnT pair
                # stationary is reused for 2 consecutive matmuls
                last = (i == order[-1])
                for half in halves:
                    wo_ps = [pav.tile([128, 512], f32, tag="av",
                                      name=f"wops{i}_{half}_{n2}")
                             for n2 in range(2)]
                    for p in range(PAIRS):
                        for n2 in range(2):
                            n = half * 2 + n2
                            nc.tensor.matmul(wo_ps[n2], aT_sb[:, p, :],
                                             wo_sb[:, p, n * 512:(n + 1) * 512],
                                             start=(p == 0), stop=(p == PAIRS - 1))
                    for n2 in range(2):
                        n = half * 2 + n2
                        sl = slice(n * 512, (n + 1) * 512)
                        if (half, n2) == (1, 1) and not last:
                            nc.scalar.copy(out_sb[:, sl], wo_ps[n2])
                        else:
                            nc.vector.tensor_copy(out_sb[:, sl], wo_ps[n2])
                        if last:
                            # drain the final tile in 512-col chunks across
                            # all three DGE queues to shorten the tail
                            eng = (nc.sync, nc.gpsimd, nc.scalar, nc.sync)[n]
                            eng.dma_start(out=y_d[i][:, sl], in_=out_sb[:, sl])
                    if not last:
                        sl2 = slice(half * 1024, (half + 1) * 1024)
                        (nc.gpsimd if i % 2 else nc.sync).dma_start(
                            out=y_d[i][:, sl2], in_=out_sb[:, sl2])
                return out_sb

            # software pipeline, batches interleaved; av(i-1) matmuls are
            # issued BEFORE phase1(i) so the reciprocal+normalize (ACT/DVE)
            # chain runs under the projection matmuls, and wo(i-1) follows
            # phase1(i) with its stationaries ready
            order = []
            for t in range(NT):
                order.append(t)
                order.append(NT + t)
            for idx, i in enumerate(order):
                phase1(i, idx)
                iprev = order[idx - 1] if idx >= 1 else None
                # av(i-1) after phase1(i): the projection window lets the
                # previous tile's exp stream drain, so the av matmuls do not
                # wait; their normalize chains hide under scores(i), and
                # wo(i-1) runs last with its stationaries ready
                if iprev is not None:
                    avps = av_alloc(iprev)
                    aT_prev = anp.tile([128, PAIRS, 128], b16, tag="att",
                                       name=f"aT{iprev}")
                    for kv in range(KVH):
                        av_pass(iprev, avps, kv)
                        av_normalize(iprev, avps, aT_prev, kv)
                    av_done(iprev)
                sgs = score_groups_of(i)
                osb = None
                final = (idx == len(order) - 1)
                for gj, g in enumerate(sgs):
                    score_group(i, *g)
                    # big tiles: wo(i-1) halves slotted into the scores
                    # stream so its matmuls absorb the ACT exp drain latency
                    if iprev is not None and len(sgs) > 4:
                        if gj == 1:
                            osb = wo_tail(iprev, aT_prev, halves=(0,))
                        elif gj == 3:
                            wo_tail(iprev, aT_prev, halves=(1,), out_sb=osb)
                            osb = "done"
                    elif iprev is not None and len(sgs) > 3 and gj == 1:
                        wo_tail(iprev, aT_prev)
                        osb = "done"
                if iprev is not None and osb != "done":
                    if osb is not None:
                        wo_tail(iprev, aT_prev, halves=(1,), out_sb=osb)
                    else:
                        wo_tail(iprev, aT_prev)
            ilast = order[-1]
            avps = av_alloc(ilast)
            aT_sb = anp.tile([128, PAIRS, 128], b16, tag="att", name=f"aT{ilast}")
            for kv in range(KVH):
                av_pass(ilast, avps, kv)
                av_normalize(ilast, avps, aT_sb, kv)
            av_done(ilast)
            wo_tail(ilast, aT_sb)

    nc.compile()
    return nc


def _prep_core(x, pos_cos, pos_sin, wq, wk, wv, wo, tp, dp):
    gh = [tp * QH + h for h in PERM]
    qcols = np.concatenate([np.arange(g * HD, (g + 1) * HD) for g in gh])
    # [group, 128, 4, 512]: D-row index = g*512 + j*128 + p
    wqr = (wq[:, qcols].astype(bf)
           .reshape(DC // 4, 4, 128, QH * HD).transpose(0, 2, 1, 3).copy())
    kvc = np.arange(tp * KVH * HD, (tp + 1) * KVH * HD)
    wkv = np.concatenate([wk[:, kvc], wv[:, kvc]], axis=1)   # [D, 256]
    wkvr = (wkv.astype(bf)
            .reshape(DC // 4, 4, 128, 2 * KVH * HD).transpose(0, 2, 1, 3).copy())
    wor = (wo[qcols, :].astype(bf)
           .reshape(PAIRS, 128, D).transpose(1, 0, 2).copy())
    xs = x[dp * BL:(dp + 1) * BL]
    xt = (xs.reshape(BL, NT, 128, DC, 128).transpose(0, 1, 4, 3, 2)
          .reshape(NQT, 128, DC, 128).astype(bf))
    cosr = (np.tile(pos_cos, (1, QH)).astype(np.float32)
            .reshape(NT, 128, QH * HD // 2).transpose(1, 0, 2).astype(bf).copy())
    sinr = (np.tile(pos_sin, (1, QH)).astype(np.float32)
            .reshape(NT, 128, QH * HD // 2).transpose(1, 0, 2).astype(bf).copy())
    maskr = np.tile(np.triu(np.ones((128, 128), np.float32)), (1, 4)).astype(bf)
    identr = np.eye(128, dtype=np.float32).astype(bf)
    return {"xt": np.ascontiguousarray(xt), "wqr": wqr, "wkvr": wkvr,
            "wor": wor, "cosr": cosr, "sinr": sinr,
            "maskr": maskr, "identr": identr}


def make_in_maps(x, pos_cos, pos_sin, wq, wk, wv, wo):
    x = np.asarray(x, np.float32)
    pos_cos = np.asarray(pos_cos, np.float32)
    pos_sin = np.asarray(pos_sin, np.float32)
    wq = np.asarray(wq, np.float32)
    wk = np.asarray(wk, np.float32)
    wv = np.asarray(wv, np.float32)
    wo = np.asarray(wo, np.float32)
    return [_prep_core(x, pos_cos, pos_sin, wq, wk, wv, wo, c % TP, c // TP)
            for c in range(TP * DP)]


def gather(results):
    y = np.empty((B, S, D), np.float32)
    for dp in range(DP):
        acc = results[dp * TP]["y"].astype(np.float32).copy()
        for t in range(1, TP):
            acc += results[dp * TP + t]["y"]
        y[dp * BL:(dp + 1) * BL] = acc.reshape(BL, S, D)
    return y


def get_nc():
    global _built
    if _built is None:
        _built = _build()
    return _built


def kernel(x, pos_cos, pos_sin, wq, wk, wv, wo):
    from concourse.bass_utils import run_bass_kernel_spmd
    nc = get_nc()
    in_maps = make_in_maps(x, pos_cos, pos_sin, wq, wk, wv, wo)
    res = run_bass_kernel_spmd(nc, in_maps, list(range(TP * DP)))
    return gather(res.results)


# revision 47
# speedup vs baseline: 1.0086x; 1.0086x over previous
# Trainium2 Bass/Tile kernel for causal GQA attention (dense_transformer).
#
# Reference computation (fp32):
#   Q = x@wq, K = x@wk, V = x@wv  (rotary on Q,K; GQA 32 q heads / 8 kv heads)
#   out = softmax(QK^T/sqrt(64), causal) @ V @ wo
#
# Sharding: tensor-parallel over heads (TP=4: 8 q heads + 2 kv heads per
# core) x data-parallel over batch (DP=2: 2 batches per core) = 8 cores.
# Each core computes a partial [2,1024,2048] output (its heads' wo
# contribution); host sums partials within each DP group.
#
# Device pipeline per core (all matmuls bf16 -> fp32 PSUM), software
# pipeline with batch-interleaved tile order:
#   av(i-1):    transposed AV: per (kv group, k-chunk) two F=256 matmuls
#     with stationary [V|ones] -> av^T psum [128, 512] whose rows 64:128
#     are 64 copies of the softmax denominator; ACT copy + fast-approx
#     reciprocal + one DVE multiply normalizes straight into the WO
#     stationary layout (no attn transposes, no per-head reciprocals).
#     Issued BEFORE phase1(i) so the normalize chain hides under proj.
#   phase1(i):  Q and fused K|V projections (contract D=2048 from
#     host-pretransposed x^T tiles), rotary on Q/K via DVE in natural
#     layout, PE-transpose Q/K to head-major [d, q] layout, V written
#     to a bf16 arena [k, chunk, 0:64=V | 64:128=ones].
#   wo(i-1):    W_O, 512-wide psum chunks, evac on ACT/DVE, DMA out.
#   scores(i):  scoresT = K^T-chunk.T @ Q^T, two heads of a pair as
#     concurrent row-tiled matmuls into separate PSUM banks; exp on ACT
#     (no max subtraction - scores are bounded ~6), diagonal causal mask
#     via gpsimd multiply; P^T parked in SBUF.
import numpy as np
import ml_dtypes

B, S, D = 4, 1024, 2048
NH, NKV, HD = 32, 8, 64
TP, DP = 4, 2
QH = NH // TP            # 8 q heads per core
KVH = NKV // TP          # 2 kv heads per core
BL = B // DP             # 2 batches per core
NT = S // 128            # 8 s-tiles per batch
NQT = BL * NT            # 16 q-tiles per core
DC = D // 128            # 16 contraction chunks for the projections
PAIRS = QH // 2          # 4 head pairs (h, h+4) packed per 128 partitions
SCALE = 1.0 / float(np.sqrt(HD))
PERM = [0, 4, 1, 5, 2, 6, 3, 7]   # local head order: pair p = (p, p+4)

bf = ml_dtypes.bfloat16

_built = None


def _build():
    from contextlib import ExitStack
    import concourse.bacc as bacc
    import concourse.tile as tile
    from concourse import mybir

    f32 = mybir.dt.float32
    b16 = mybir.dt.bfloat16
    Exp = mybir.ActivationFunctionType.Exp

    nc = bacc.Bacc("TRN2", target_bir_lowering=False, debug=False,
                   num_devices=TP * DP)

    xt_d = nc.dram_tensor("xt", [NQT, 128, DC, 128], b16, kind="ExternalInput").ap()
    # weights grouped 4 contraction-chunks per DMA: [group, 128, 4, N]
    wq_d = nc.dram_tensor("wqr", [DC // 4, 128, 4, QH * HD], b16, kind="ExternalInput").ap()
    wkv_d = nc.dram_tensor("wkvr", [DC // 4, 128, 4, 2 * KVH * HD], b16, kind="ExternalInput").ap()
    wo_d = nc.dram_tensor("wor", [128, PAIRS, D], b16, kind="ExternalInput").ap()
    cos_d = nc.dram_tensor("cosr", [128, NT, QH * HD // 2], b16, kind="ExternalInput").ap()
    sin_d = nc.dram_tensor("sinr", [128, NT, QH * HD // 2], b16, kind="ExternalInput").ap()
    mask_d = nc.dram_tensor("maskr", [128, 512], b16, kind="ExternalInput").ap()
    id_d = nc.dram_tensor("identr", [128, 128], b16, kind="ExternalInput").ap()
    y_d = nc.dram_tensor("y", [NQT, 128, D], b16, kind="ExternalOutput").ap()

    with tile.TileContext(nc) as tc:
        with ExitStack() as ctx:
            singles = ctx.enter_context(tc.tile_pool(name="singles", bufs=1))
            # PSUM: 8 banks: 3 proj + 2 scores/tq + 3 av/wo
            pp = ctx.enter_context(tc.tile_pool(name="pp", bufs=3, space="PSUM"))
            psc = ctx.enter_context(tc.tile_pool(name="psc", bufs=3, space="PSUM"))
            pav = ctx.enter_context(tc.tile_pool(name="pav", bufs=2, space="PSUM"))
            xtp = ctx.enter_context(tc.tile_pool(name="xtp", bufs=5))
            rot = ctx.enter_context(tc.tile_pool(name="rot", bufs=2))
            rtmp = ctx.enter_context(tc.tile_pool(name="rtmp", bufs=4))
            persist = ctx.enter_context(tc.tile_pool(name="persist", bufs=1))
            ptp = ctx.enter_context(tc.tile_pool(name="ptp", bufs=1))
            anp = ctx.enter_context(tc.tile_pool(name="anp", bufs=3))
            outp = ctx.enter_context(tc.tile_pool(name="outp", bufs=3))

            mask_sb = singles.tile([128, 512], b16)
            ident_sb = singles.tile([128, 128], b16)
            wq_g = [singles.tile([128, 4, QH * HD], b16, name=f"wq{g}")
                    for g in range(DC // 4)]
            wkv_g = [singles.tile([128, 4, 2 * KVH * HD], b16, name=f"wkv{g}")
                     for g in range(DC // 4)]
            cos_sb = singles.tile([128, NT, QH * HD // 2], b16)
            sin_sb = singles.tile([128, NT, QH * HD // 2], b16)
            wo_sb = singles.tile([128, PAIRS, D], b16)
            # V arena per (local batch, kv head): [k, chunk, 0:64=V|64:128=1]
            varena = [[singles.tile([128, NT, 128], b16, name=f"va{b}_{kv}")
                       for kv in range(KVH)] for b in range(BL)]
            for b_ in range(BL):
                for kv in range(KVH):
                    nc.vector.memset(varena[b_][kv][:, :, HD:128], 1.0)

            xt_sbs = {}

            def load_xt(i, eng):
                xt_sb = xtp.tile([128, DC, 128], b16, tag="xt", name=f"xt{i}")
                eng.dma_start(out=xt_sb, in_=xt_d[i])
                xt_sbs[i] = xt_sb

            def load_xt_split(i, eng1, eng2, cut=4):
                xt_sb = xtp.tile([128, DC, 128], b16, tag="xt", name=f"xt{i}")
                eng1.dma_start(out=xt_sb[:, 0:cut, :], in_=xt_d[i][:, 0:cut, :])
                eng2.dma_start(out=xt_sb[:, cut:DC, :], in_=xt_d[i][:, cut:DC, :])
                xt_sbs[i] = xt_sb

            # Startup-critical bytes split finely across the DGE queues so
            # each queue's FIRST transfer is exactly what the PE needs first.
            nc.sync.dma_start(out=wq_g[0][:, 0, :], in_=wq_d[0][:, 0, :])
            nc.gpsimd.dma_start(out=wkv_g[0][:, 0:2, :], in_=wkv_d[0][:, 0:2, :])
            load_xt_split(0, nc.scalar, nc.scalar)
            nc.sync.dma_start(out=wq_g[0][:, 1:4, :], in_=wq_d[0][:, 1:4, :])
            nc.gpsimd.dma_start(out=wkv_g[0][:, 2:4, :], in_=wkv_d[0][:, 2:4, :])
            nc.gpsimd.dma_start(out=cos_sb[:, 0:2, :], in_=cos_d[:, 0:2, :])
            nc.gpsimd.dma_start(out=sin_sb[:, 0:2, :], in_=sin_d[:, 0:2, :])
            nc.sync.dma_start(out=wq_g[1], in_=wq_d[1])
            nc.gpsimd.dma_start(out=ident_sb, in_=id_d)
            nc.scalar.dma_start(out=wkv_g[1], in_=wkv_d[1])
            load_xt(NT, nc.sync)
            nc.gpsimd.dma_start(out=mask_sb, in_=mask_d)
            nc.sync.dma_start(out=wq_g[2], in_=wq_d[2])
            nc.gpsimd.dma_start(out=wkv_g[2], in_=wkv_d[2])
            load_xt(1, nc.scalar)
            nc.sync.dma_start(out=wq_g[3], in_=wq_d[3])
            nc.scalar.dma_start(out=wkv_g[3], in_=wkv_d[3])
            nc.sync.dma_start(out=wo_sb, in_=wo_d)

            qt_tiles = {}
            kt_tiles = {}
            pt_tiles = {}

            def phase1(i, idx):
                bl, t = divmod(i, NT)
                # ---------- QKV projection ----------
                if idx == 0 and idx + 4 < len(order):
                    # halves on different queues: the consumer reads chunks
                    # sequentially, so the late half has ~2.6us of slack
                    load_xt_split(order[idx + 3], nc.gpsimd, nc.sync, cut=8)
                    load_xt_split(order[idx + 4], nc.sync, nc.gpsimd, cut=8)
                elif idx + 4 < len(order):
                    load_xt(order[idx + 4],
                            nc.sync if idx % 2 == 0 else nc.gpsimd)
                if idx == 1:
                    nc.gpsimd.dma_start(out=cos_sb[:, 2:4, :], in_=cos_d[:, 2:4, :])
                    nc.gpsimd.dma_start(out=sin_sb[:, 2:4, :], in_=sin_d[:, 2:4, :])
                if idx == 4:
                    nc.gpsimd.dma_start(out=cos_sb[:, 4:6, :], in_=cos_d[:, 4:6, :])
                    nc.gpsimd.dma_start(out=sin_sb[:, 4:6, :], in_=sin_d[:, 4:6, :])
                if idx == 8:
                    nc.gpsimd.dma_start(out=cos_sb[:, 6:NT, :], in_=cos_d[:, 6:NT, :])
                    nc.gpsimd.dma_start(out=sin_sb[:, 6:NT, :], in_=sin_d[:, 6:NT, :])
                xt_sb = xt_sbs.pop(i)
                q_ps = pp.tile([128, QH * HD], f32, tag="pp", name=f"qps{i}")
                kv_ps = pp.tile([128, 2 * KVH * HD], f32, tag="pp", name=f"kvps{i}")
                for c in range(DC):
                    st, sp = (c == 0), (c == DC - 1)
                    g, j = divmod(c, 4)
                    nc.tensor.matmul(q_ps, xt_sb[:, c, :], wq_g[g][:, j, :],
                                     start=st, stop=sp)
                    nc.tensor.matmul(kv_ps, xt_sb[:, c, :], wkv_g[g][:, j, :],
                                     start=st, stop=sp)

                # ---------- rotary (natural layout, pairs on free dim) ----
                c_sl = cos_sb[:, t, :]
                s_sl = sin_sb[:, t, :]
                qrot = rot.tile([128, QH * HD], b16, tag="qrot", name=f"qr{i}")
                qv = qrot.rearrange("p (n two) -> p two n", two=2)
                qp = q_ps.rearrange("p (n two) -> p two n", two=2)
                t1 = rtmp.tile([128, QH * HD // 2], f32, tag="t1", name=f"t1a{i}")
                t2 = rtmp.tile([128, QH * HD // 2], f32, tag="t2", name=f"t2a{i}")
                nc.vector.tensor_mul(t1, qp[:, 0, :], c_sl)
                nc.vector.tensor_mul(t2, qp[:, 1, :], s_sl)
                nc.vector.tensor_sub(qv[:, 0, :], t1, t2)
                t3 = rtmp.tile([128, QH * HD // 2], f32, tag="t1", name=f"t1b{i}")
                t4 = rtmp.tile([128, QH * HD // 2], f32, tag="t2", name=f"t2b{i}")
                nc.vector.tensor_mul(t3, qp[:, 0, :], s_sl)
                nc.vector.tensor_mul(t4, qp[:, 1, :], c_sl)
                nc.vector.tensor_add(qv[:, 1, :], t3, t4)

                ck_sl = cos_sb[:, t, 0:KVH * HD // 2]
                sk_sl = sin_sb[:, t, 0:KVH * HD // 2]
                krot = rot.tile([128, KVH * HD], b16, tag="krot", name=f"kr{i}")
                kv_ = krot.rearrange("p (n two) -> p two n", two=2)
                kp = kv_ps[:, 0:KVH * HD].rearrange("p (n two) -> p two n", two=2)
                u1 = rtmp.tile([128, KVH * HD // 2], f32, tag="u1", name=f"u1a{i}")
                u2 = rtmp.tile([128, KVH * HD // 2], f32, tag="u2", name=f"u2a{i}")
                nc.vector.tensor_mul(u1, kp[:, 0, :], ck_sl)
                nc.vector.tensor_mul(u2, kp[:, 1, :], sk_sl)
                nc.vector.tensor_sub(kv_[:, 0, :], u1, u2)
                u3 = rtmp.tile([128, KVH * HD // 2], f32, tag="u1", name=f"u1b{i}")
                u4 = rtmp.tile([128, KVH * HD // 2], f32, tag="u2", name=f"u2b{i}")
                nc.vector.tensor_mul(u3, kp[:, 0, :], sk_sl)
                nc.vector.tensor_mul(u4, kp[:, 1, :], ck_sl)
                nc.vector.tensor_add(kv_[:, 1, :], u3, u4)

                # ---------- V -> bf16 arena (ones cols are static) ----------
                voff = KVH * HD
                for kv in range(KVH):
                    nc.vector.tensor_copy(
                        varena[bl][kv][:, t, 0:HD],
                        kv_ps[:, voff + kv * HD:voff + (kv + 1) * HD])

                # ---------- PE transposes to [d, q]: 4 Q pairs + K into one
                # psum bank, evacuated with a single wide DVE copy ----------
                tq_ps = psc.tile([128, PAIRS + 1, 128], b16, tag="sc",
                                 name=f"tq{i}")
                for j in range(PAIRS):
                    nc.tensor.transpose(tq_ps[:, j, :],
                                        qrot[:, j * 128:(j + 1) * 128], ident_sb)
                nc.tensor.transpose(tq_ps[:, PAIRS, :], krot, ident_sb)
                qkt_sb = persist.tile([128, PAIRS + 1, 128], b16, tag="qt",
                                      bufs=2 * NT + 4, name=f"qkt{i}")
                nc.vector.tensor_copy(qkt_sb, tq_ps)
                qt_tiles[i] = qkt_sb
                kt_tiles[i] = qkt_sb[:, PAIRS, :]

            def score_group(i, pg, g0):
                bl, t = divmod(i, NT)
                ib = bl * NT
                nkc = t + 1
                # two pairs per matmul: moving = [64, 2 pairs, 128 q] (256
                # cols); A/B halves alternate PE row-tiles so ldweights hide
                qkt = qt_tiles[i]
                qtt2A = qkt[0:64, 2 * pg:2 * pg + 2, :].rearrange(
                    "p a b -> p (a b)")
                qtt2B = qkt[64:128, 2 * pg:2 * pg + 2, :].rearrange(
                    "p a b -> p (a b)")
                gk = min(2, nkc - g0)
                scA = psc.tile([128, 512], f32, tag="sc",
                               name=f"scA{i}_{pg}_{g0}")
                scB = psc.tile([128, 512], f32, tag="sc",
                               name=f"scB{i}_{pg}_{g0}")
                for gi in range(gk):
                    ktt = kt_tiles[ib + g0 + gi]
                    o = gi * 256
                    nc.tensor.matmul(scA[:, o:o + 256],
                                     ktt[0:64, :], qtt2A,
                                     start=True, stop=True)
                    nc.tensor.matmul(scB[:, o:o + 256],
                                     ktt[64:128, :], qtt2B,
                                     start=True, stop=True)
                if (i, g0) not in pt_tiles:
                    pt_tiles[(i, g0)] = (
                        ptp.tile([128, 2, 512], b16, tag="pt", bufs=20,
                                 name=f"ptA{i}_{g0}"),
                        ptp.tile([128, 2, 512], b16, tag="pt", bufs=20,
                                 name=f"ptB{i}_{g0}"))
                ptA2, ptB2 = pt_tiles[(i, g0)]
                ptA, ptB = ptA2[:, pg, :], ptB2[:, pg, :]
                nc.scalar.activation(ptA[:, 0:gk * 256], scA[:, 0:gk * 256],
                                     Exp, scale=SCALE)
                nc.scalar.activation(ptB[:, 0:gk * 256], scB[:, 0:gk * 256],
                                     Exp, scale=SCALE)
                if g0 + gk == nkc:   # diagonal chunk is last
                    o = (gk - 1) * 256
                    nc.gpsimd.tensor_mul(ptA[:, o:o + 256],
                                         ptA[:, o:o + 256],
                                         mask_sb[:, 0:256])
                    nc.gpsimd.tensor_mul(ptB[:, o:o + 256],
                                         ptB[:, o:o + 256],
                                         mask_sb[:, 0:256])

            def score_groups_of(i):
                bl, t = divmod(i, NT)
                return [(pg, g0) for pg in range(PAIRS // 2)
                        for g0 in range(0, t + 1, 2)]

            def av_alloc(i):
                return [pav.tile([128, 512], f32, tag="av", name=f"av{i}_{kv}")
                        for kv in range(KVH)]

            def av_pass(i, avps, kv):
                # transposed AV: per (kv, kc) ONE F=512 matmul, stationary
                # [V|ones]; moving = both pg groups' P^T for chunk kc via a
                # 3D strided AP [128, (pg 2, stride 512), 256]
                bl, t = divmod(i, NT)
                nkc = t + 1
                avp = avps[kv]
                for kc in range(nkc):
                    g0, gi = (kc // 2) * 2, kc % 2
                    ptA2, ptB2 = pt_tiles[(i, g0)]
                    pt_ = ptA2 if kv == 0 else ptB2
                    o = gi * 256
                    nc.tensor.matmul(avp, varena[bl][kv][:, kc, :],
                                     pt_[:, :, o:o + 256],
                                     start=(kc == 0), stop=(kc == nkc - 1))

            def av_done(i):
                bl, t = divmod(i, NT)
                for g02 in range(0, t + 1, 2):
                    pt_tiles.pop((i, g02), None)

            def av_normalize(i, avps, aT_sb, kv):
                # rows 64:128 of avp = 64 copies of the softmax denominator
                # (approx-recip is a custom DVE op: needs SBUF input)
                avp = avps[kv]
                denb = anp.tile([64, 512], f32, tag="denb",
                                name=f"db{i}_{kv}")
                nc.scalar.copy(denb, avp[64:128, :])
                recip = anp.tile([64, 512], f32, tag="recip",
                                 name=f"rc{i}_{kv}")
                nc.vector.reciprocal_approx_fast(out=recip, in_=denb)
                nc.vector.tensor_mul(
                    aT_sb[kv * 64:(kv + 1) * 64, :, :].rearrange(
                        "p a b -> p (a b)"),
                    avp[0:64, :], recip)

            def wo_tail(i, aT_sb, halves=(0, 1), out_sb=None):
                if out_sb is None:
                    out_sb = outp.tile([128, D], b16, tag="out", name=f"o{i}")
                # two 512-wide output chunks in flight so each attnT pair
                # stationary is reused for 2 consecutive matmuls
                last = (i == order[-1])
                for half in halves:
                    wo_ps = [pav.tile([128, 512], f32, tag="av",
                                      name=f"wops{i}_{half}_{n2}")
                             for n2 in range(2)]
                    for p in range(PAIRS):
                        for n2 in range(2):
                            n = half * 2 + n2
                            nc.tensor.matmul(wo_ps[n2], aT_sb[:, p, :],
                                             wo_sb[:, p, n * 512:(n + 1) * 512],
                                             start=(p == 0), stop=(p == PAIRS - 1))
                    for n2 in range(2):
                        n = half * 2 + n2
                        sl = slice(n * 512, (n + 1) * 512)
                        if (half, n2) == (1, 1) and not last:
                            nc.scalar.copy(out_sb[:, sl], wo_ps[n2])
                        else:
                            nc.vector.tensor_copy(out_sb[:, sl], wo_ps[n2])
                        if last:
                            # drain the final tile in 512-col chunks across
                            # all three DGE queues to shorten the tail
                            eng = (nc.sync, nc.gpsimd, nc.scalar, nc.sync)[n]
                            eng.dma_start(out=y_d[i][:, sl], in_=out_sb[:, sl])
                    if not last:
                        sl2 = slice(half * 1024, (half + 1) * 1024)
                        (nc.gpsimd if i % 2 else nc.sync).dma_start(
                            out=y_d[i][:, sl2], in_=out_sb[:, sl2])
                return out_sb

            # software pipeline, batches interleaved; av(i-1) matmuls are
            # issued BEFORE phase1(i) so the reciprocal+normalize (ACT/DVE)
            # chain runs under the projection matmuls, and wo(i-1) follows
            # phase1(i) with its stationaries ready
            order = []
            for t in range(NT):
                order.append(t)
                order.append(NT + t)
            for idx, i in enumerate(order):
                phase1(i, idx)
                iprev = order[idx - 1] if idx >= 1 else None
                # av(i-1) after phase1(i): the projection window lets the
                # previous tile's exp stream drain, so the av matmuls do not
                # wait; their normalize chains hide under scores(i), and
                # wo(i-1) runs last with its stationaries ready
                if iprev is not None:
                    avps = av_alloc(iprev)
                    aT_prev = anp.tile([128, PAIRS, 128], b16, tag="att",
                                       name=f"aT{iprev}")
                    for kv in range(KVH):
                        av_pass(iprev, avps, kv)
                        av_normalize(iprev, avps, aT_prev, kv)
                    av_done(iprev)
                sgs = score_groups_of(i)
                osb = None
                final = (idx == len(order) - 1)
                for gj, g in enumerate(sgs):
                    score_group(i, *g)
                    # big tiles: wo(i-1) halves slotted into the scores
                    # stream so its matmuls absorb the ACT exp drain latency
                    if iprev is not None and len(sgs) > 4:
                        if gj == 1:
                            osb = wo_tail(iprev, aT_prev, halves=(0,))
                        elif gj == 3:
                            wo_tail(iprev, aT_prev, halves=(1,), out_sb=osb)
                            osb = "done"
                    elif iprev is not None and len(sgs) > 3 and gj == 1:
                        wo_tail(iprev, aT_prev)
                        osb = "done"
                if iprev is not None and osb != "done":
                    if osb is not None:
                        wo_tail(iprev, aT_prev, halves=(1,), out_sb=osb)
                    else:
                        wo_tail(iprev, aT_prev)
            ilast = order[-1]
            avps = av_alloc(ilast)
            aT_sb = anp.tile([128, PAIRS, 128], b16, tag="att", name=f"aT{ilast}")
            for kv in range(KVH):
                av_pass(ilast, avps, kv)
                av_normalize(ilast, avps, aT_sb, kv)
            av_done(ilast)
            wo_tail(ilast, aT_sb)

    nc.compile()
    return nc


def _prep_core(x, pos_cos, pos_sin, wq, wk, wv, wo, tp, dp):
    gh = [tp * QH + h for h in PERM]
    qcols = np.concatenate([np.arange(g * HD, (g + 1) * HD) for g in gh])
    # [group, 128, 4, 512]: D-row index = g*512 + j*128 + p
    wqr = (wq[:, qcols].astype(bf)
           .reshape(DC // 4, 4, 128, QH * HD).transpose(0, 2, 1, 3).copy())
    kvc = np.arange(tp * KVH * HD, (tp + 1) * KVH * HD)
    wkv = np.concatenate([wk[:, kvc], wv[:, kvc]], axis=1)   # [D, 256]
    wkvr = (wkv.astype(bf)
            .reshape(DC // 4, 4, 128, 2 * KVH * HD).transpose(0, 2, 1, 3).copy())
    wor = (wo[qcols, :].astype(bf)
           .reshape(PAIRS, 128, D).transpose(1, 0, 2).copy())
    xs = x[dp * BL:(dp + 1) * BL]
    xt = (xs.reshape(BL, NT, 128, DC, 128).transpose(0, 1, 4, 3, 2)
          .reshape(NQT, 128, DC, 128).astype(bf))
    cosr = (np.tile(pos_cos, (1, QH)).astype(np.float32)
            .reshape(NT, 128, QH * HD // 2).transpose(1, 0, 2).astype(bf).copy())
    sinr = (np.tile(pos_sin, (1, QH)).astype(np.float32)
            .reshape(NT, 128, QH * HD // 2).transpose(1, 0, 2).astype(bf).copy())
    maskr = np.tile(np.triu(np.ones((128, 128), np.float32)), (1, 4)).astype(bf)
    identr = np.eye(128, dtype=np.float32).astype(bf)
    return {"xt": np.ascontiguousarray(xt), "wqr": wqr, "wkvr": wkvr,
            "wor": wor, "cosr": cosr, "sinr": sinr,
            "maskr": maskr, "identr": identr}


def make_in_maps(x, pos_cos, pos_sin, wq, wk, wv, wo):
    x = np.asarray(x, np.float32)
    pos_cos = np.asarray(pos_cos, np.float32)
    pos_sin = np.asarray(pos_sin, np.float32)
    wq = np.asarray(wq, np.float32)
    wk = np.asarray(wk, np.float32)
    wv = np.asarray(wv, np.float32)
    wo = np.asarray(wo, np.float32)
    return [_prep_core(x, pos_cos, pos_sin, wq, wk, wv, wo, c % TP, c // TP)
            for c in range(TP * DP)]


def gather(results):
    y = np.empty((B, S, D), np.float32)
    for dp in range(DP):
        acc = results[dp * TP]["y"].astype(np.float32).copy()
        for t in range(1, TP):
            acc += results[dp * TP + t]["y"]
        y[dp * BL:(dp + 1) * BL] = acc.reshape(BL, S, D)
    return y


def get_nc():
    global _built
    if _built is None:
        _built = _build()
    return _built


def kernel(x, pos_cos, pos_sin, wq, wk, wv, wo):
    from concourse.bass_utils import run_bass_kernel_spmd
    nc = get_nc()
    in_maps = make_in_maps(x, pos_cos, pos_sin, wq, wk, wv, wo)
    res = run_bass_kernel_spmd(nc, in_maps, list(range(TP * DP)))
    return gather(res.results)


# revision 48
# speedup vs baseline: 1.0129x; 1.0042x over previous
# Trainium2 Bass/Tile kernel for causal GQA attention (dense_transformer).
#
# Reference computation (fp32):
#   Q = x@wq, K = x@wk, V = x@wv  (rotary on Q,K; GQA 32 q heads / 8 kv heads)
#   out = softmax(QK^T/sqrt(64), causal) @ V @ wo
#
# Sharding: tensor-parallel over heads (TP=4: 8 q heads + 2 kv heads per
# core) x data-parallel over batch (DP=2: 2 batches per core) = 8 cores.
# Each core computes a partial [2,1024,2048] output (its heads' wo
# contribution); host sums partials within each DP group.
#
# Device pipeline per core (all matmuls bf16 -> fp32 PSUM), software
# pipeline with batch-interleaved tile order:
#   av(i-1):    transposed AV: per (kv group, k-chunk) two F=256 matmuls
#     with stationary [V|ones] -> av^T psum [128, 512] whose rows 64:128
#     are 64 copies of the softmax denominator; ACT copy + fast-approx
#     reciprocal + one DVE multiply normalizes straight into the WO
#     stationary layout (no attn transposes, no per-head reciprocals).
#     Issued BEFORE phase1(i) so the normalize chain hides under proj.
#   phase1(i):  Q and fused K|V projections (contract D=2048 from
#     host-pretransposed x^T tiles), rotary on Q/K via DVE in natural
#     layout, PE-transpose Q/K to head-major [d, q] layout, V written
#     to a bf16 arena [k, chunk, 0:64=V | 64:128=ones].
#   wo(i-1):    W_O, 512-wide psum chunks, evac on ACT/DVE, DMA out.
#   scores(i):  scoresT = K^T-chunk.T @ Q^T, two heads of a pair as
#     concurrent row-tiled matmuls into separate PSUM banks; exp on ACT
#     (no max subtraction - scores are bounded ~6), diagonal causal mask
#     via gpsimd multiply; P^T parked in SBUF.
import numpy as np
import ml_dtypes

B, S, D = 4, 1024, 2048
NH, NKV, HD = 32, 8, 64
TP, DP = 4, 2
QH = NH // TP            # 8 q heads per core
KVH = NKV // TP          # 2 kv heads per core
BL = B // DP             # 2 batches per core
NT = S // 128            # 8 s-tiles per batch
NQT = BL * NT            # 16 q-tiles per core
DC = D // 128            # 16 contraction chunks for the projections
PAIRS = QH // 2          # 4 head pairs (h, h+4) packed per 128 partitions
SCALE = 1.0 / float(np.sqrt(HD))
PERM = [0, 4, 1, 5, 2, 6, 3, 7]   # local head order: pair p = (p, p+4)

bf = ml_dtypes.bfloat16

_built = None


def _build():
    from contextlib import ExitStack
    import concourse.bacc as bacc
    import concourse.tile as tile
    from concourse import mybir

    f32 = mybir.dt.float32
    b16 = mybir.dt.bfloat16
    Exp = mybir.ActivationFunctionType.Exp

    nc = bacc.Bacc("TRN2", target_bir_lowering=False, debug=False,
                   num_devices=TP * DP)

    xt_d = nc.dram_tensor("xt", [NQT, 128, DC, 128], b16, kind="ExternalInput").ap()
    # weights grouped 4 contraction-chunks per DMA: [group, 128, 4, N]
    wq_d = nc.dram_tensor("wqr", [DC // 4, 128, 4, QH * HD], b16, kind="ExternalInput").ap()
    wkv_d = nc.dram_tensor("wkvr", [DC // 4, 128, 4, 2 * KVH * HD], b16, kind="ExternalInput").ap()
    wo_d = nc.dram_tensor("wor", [128, PAIRS, D], b16, kind="ExternalInput").ap()
    cos_d = nc.dram_tensor("cosr", [128, NT, QH * HD // 2], b16, kind="ExternalInput").ap()
    sin_d = nc.dram_tensor("sinr", [128, NT, QH * HD // 2], b16, kind="ExternalInput").ap()
    mask_d = nc.dram_tensor("maskr", [128, 512], b16, kind="ExternalInput").ap()
    id_d = nc.dram_tensor("identr", [128, 128], b16, kind="ExternalInput").ap()
    y_d = nc.dram_tensor("y", [NQT, 128, D], b16, kind="ExternalOutput").ap()

    with tile.TileContext(nc) as tc:
        with ExitStack() as ctx:
            singles = ctx.enter_context(tc.tile_pool(name="singles", bufs=1))
            # PSUM: 8 banks: 3 proj + 2 scores/tq + 3 av/wo
            pp = ctx.enter_context(tc.tile_pool(name="pp", bufs=3, space="PSUM"))
            psc = ctx.enter_context(tc.tile_pool(name="psc", bufs=3, space="PSUM"))
            pav = ctx.enter_context(tc.tile_pool(name="pav", bufs=2, space="PSUM"))
            xtp = ctx.enter_context(tc.tile_pool(name="xtp", bufs=5))
            rot = ctx.enter_context(tc.tile_pool(name="rot", bufs=2))
            rtmp = ctx.enter_context(tc.tile_pool(name="rtmp", bufs=4))
            persist = ctx.enter_context(tc.tile_pool(name="persist", bufs=1))
            ptp = ctx.enter_context(tc.tile_pool(name="ptp", bufs=1))
            anp = ctx.enter_context(tc.tile_pool(name="anp", bufs=3))
            outp = ctx.enter_context(tc.tile_pool(name="outp", bufs=3))

            mask_sb = singles.tile([128, 512], b16)
            ident_sb = singles.tile([128, 128], b16)
            wq_g = [singles.tile([128, 4, QH * HD], b16, name=f"wq{g}")
                    for g in range(DC // 4)]
            wkv_g = [singles.tile([128, 4, 2 * KVH * HD], b16, name=f"wkv{g}")
                     for g in range(DC // 4)]
            cos_sb = singles.tile([128, NT, QH * HD // 2], b16)
            sin_sb = singles.tile([128, NT, QH * HD // 2], b16)
            wo_sb = singles.tile([128, PAIRS, D], b16)
            # V arena per (local batch, kv head): [k, chunk, 0:64=V|64:128=1]
            varena = [[singles.tile([128, NT, 128], b16, name=f"va{b}_{kv}")
                       for kv in range(KVH)] for b in range(BL)]
            for b_ in range(BL):
                for kv in range(KVH):
                    nc.vector.memset(varena[b_][kv][:, :, HD:128], 1.0)

            xt_sbs = {}

            def load_xt(i, eng):
                xt_sb = xtp.tile([128, DC, 128], b16, tag="xt", name=f"xt{i}")
                eng.dma_start(out=xt_sb, in_=xt_d[i])
                xt_sbs[i] = xt_sb

            def load_xt_split(i, eng1, eng2, cut=4):
                xt_sb = xtp.tile([128, DC, 128], b16, tag="xt", name=f"xt{i}")
                eng1.dma_start(out=xt_sb[:, 0:cut, :], in_=xt_d[i][:, 0:cut, :])
                eng2.dma_start(out=xt_sb[:, cut:DC, :], in_=xt_d[i][:, cut:DC, :])
                xt_sbs[i] = xt_sb

            # Startup-critical bytes split finely across the DGE queues so
            # each queue's FIRST transfer is exactly what the PE needs first.
            nc.sync.dma_start(out=wq_g[0][:, 0, :], in_=wq_d[0][:, 0, :])
            nc.gpsimd.dma_start(out=wkv_g[0][:, 0:2, :], in_=wkv_d[0][:, 0:2, :])
            load_xt_split(0, nc.scalar, nc.scalar)
            nc.sync.dma_start(out=wq_g[0][:, 1:4, :], in_=wq_d[0][:, 1:4, :])
            nc.gpsimd.dma_start(out=wkv_g[0][:, 2:4, :], in_=wkv_d[0][:, 2:4, :])
            nc.gpsimd.dma_start(out=cos_sb[:, 0:2, :], in_=cos_d[:, 0:2, :])
            nc.gpsimd.dma_start(out=sin_sb[:, 0:2, :], in_=sin_d[:, 0:2, :])
            nc.sync.dma_start(out=wq_g[1], in_=wq_d[1])
            nc.gpsimd.dma_start(out=ident_sb, in_=id_d)
            nc.scalar.dma_start(out=wkv_g[1], in_=wkv_d[1])
            load_xt(NT, nc.sync)
            nc.gpsimd.dma_start(out=mask_sb, in_=mask_d)
            nc.sync.dma_start(out=wq_g[2], in_=wq_d[2])
            nc.gpsimd.dma_start(out=wkv_g[2], in_=wkv_d[2])
            load_xt(1, nc.scalar)
            nc.sync.dma_start(out=wq_g[3], in_=wq_d[3])
            nc.scalar.dma_start(out=wkv_g[3], in_=wkv_d[3])
            nc.sync.dma_start(out=wo_sb, in_=wo_d)

            qt_tiles = {}
            kt_tiles = {}
            pt_tiles = {}

            def phase1(i, idx):
                bl, t = divmod(i, NT)
                # ---------- QKV projection ----------
                if idx == 0 and idx + 4 < len(order):
                    # halves on different queues: the consumer reads chunks
                    # sequentially, so the late half has ~2.6us of slack
                    load_xt_split(order[idx + 3], nc.gpsimd, nc.sync, cut=8)
                    load_xt_split(order[idx + 4], nc.sync, nc.gpsimd, cut=8)
                elif idx + 4 < len(order):
                    load_xt(order[idx + 4],
                            nc.sync if idx % 2 == 0 else nc.gpsimd)
                if idx == 1:
                    nc.gpsimd.dma_start(out=cos_sb[:, 2:4, :], in_=cos_d[:, 2:4, :])
                    nc.gpsimd.dma_start(out=sin_sb[:, 2:4, :], in_=sin_d[:, 2:4, :])
                if idx == 4:
                    nc.gpsimd.dma_start(out=cos_sb[:, 4:6, :], in_=cos_d[:, 4:6, :])
                    nc.gpsimd.dma_start(out=sin_sb[:, 4:6, :], in_=sin_d[:, 4:6, :])
                if idx == 8:
                    nc.gpsimd.dma_start(out=cos_sb[:, 6:NT, :], in_=cos_d[:, 6:NT, :])
                    nc.gpsimd.dma_start(out=sin_sb[:, 6:NT, :], in_=sin_d[:, 6:NT, :])
                xt_sb = xt_sbs.pop(i)
                q_ps = pp.tile([128, QH * HD], f32, tag="pp", name=f"qps{i}")
                kv_ps = pp.tile([128, 2 * KVH * HD], f32, tag="pp", name=f"kvps{i}")
                for c in range(DC):
                    st, sp = (c == 0), (c == DC - 1)
                    g, j = divmod(c, 4)
                    nc.tensor.matmul(q_ps, xt_sb[:, c, :], wq_g[g][:, j, :],
                                     start=st, stop=sp)
                    nc.tensor.matmul(kv_ps, xt_sb[:, c, :], wkv_g[g][:, j, :],
                                     start=st, stop=sp)

                # ---------- rotary (natural layout, pairs on free dim) ----
                c_sl = cos_sb[:, t, :]
                s_sl = sin_sb[:, t, :]
                qrot = rot.tile([128, QH * HD], b16, tag="qrot", name=f"qr{i}")
                qv = qrot.rearrange("p (n two) -> p two n", two=2)
                qp = q_ps.rearrange("p (n two) -> p two n", two=2)
                t1 = rtmp.tile([128, QH * HD // 2], f32, tag="t1", name=f"t1a{i}")
                t2 = rtmp.tile([128, QH * HD // 2], f32, tag="t2", name=f"t2a{i}")
                nc.vector.tensor_mul(t1, qp[:, 0, :], c_sl)
                nc.vector.tensor_mul(t2, qp[:, 1, :], s_sl)
                nc.vector.tensor_sub(qv[:, 0, :], t1, t2)
                t3 = rtmp.tile([128, QH * HD // 2], f32, tag="t1", name=f"t1b{i}")
                t4 = rtmp.tile([128, QH * HD // 2], f32, tag="t2", name=f"t2b{i}")
                nc.vector.tensor_mul(t3, qp[:, 0, :], s_sl)
                nc.vector.tensor_mul(t4, qp[:, 1, :], c_sl)
                nc.vector.tensor_add(qv[:, 1, :], t3, t4)

                ck_sl = cos_sb[:, t, 0:KVH * HD // 2]
                sk_sl = sin_sb[:, t, 0:KVH * HD // 2]
                krot = rot.tile([128, KVH * HD], b16, tag="krot", name=f"kr{i}")
                kv_ = krot.rearrange("p (n two) -> p two n", two=2)
                kp = kv_ps[:, 0:KVH * HD].rearrange("p (n two) -> p two n", two=2)
                u1 = rtmp.tile([128, KVH * HD // 2], f32, tag="u1", name=f"u1a{i}")
                u2 = rtmp.tile([128, KVH * HD // 2], f32, tag="u2", name=f"u2a{i}")
                nc.vector.tensor_mul(u1, kp[:, 0, :], ck_sl)
                nc.vector.tensor_mul(u2, kp[:, 1, :], sk_sl)
                nc.vector.tensor_sub(kv_[:, 0, :], u1, u2)
                u3 = rtmp.tile([128, KVH * HD // 2], f32, tag="u1", name=f"u1b{i}")
                u4 = rtmp.tile([128, KVH * HD // 2], f32, tag="u2", name=f"u2b{i}")
                nc.vector.tensor_mul(u3, kp[:, 0, :], sk_sl)
                nc.vector.tensor_mul(u4, kp[:, 1, :], ck_sl)
                nc.vector.tensor_add(kv_[:, 1, :], u3, u4)

                # ---------- V -> bf16 arena (ones cols are static) ----------
                voff = KVH * HD
                for kv in range(KVH):
                    nc.vector.tensor_copy(
                        varena[bl][kv][:, t, 0:HD],
                        kv_ps[:, voff + kv * HD:voff + (kv + 1) * HD])

                # ---------- PE transposes to [d, q]: 4 Q pairs + K into one
                # psum bank, evacuated with a single wide DVE copy ----------
                tq_ps = pav.tile([128, PAIRS + 1, 128], b16, tag="av",
                                 name=f"tq{i}")
                for j in range(PAIRS):
                    nc.tensor.transpose(tq_ps[:, j, :],
                                        qrot[:, j * 128:(j + 1) * 128], ident_sb)
                nc.tensor.transpose(tq_ps[:, PAIRS, :], krot, ident_sb)
                qkt_sb = persist.tile([128, PAIRS + 1, 128], b16, tag="qt",
                                      bufs=2 * NT + 4, name=f"qkt{i}")
                nc.vector.tensor_copy(qkt_sb, tq_ps)
                qt_tiles[i] = qkt_sb
                kt_tiles[i] = qkt_sb[:, PAIRS, :]

            def score_group(i, pg, g0):
                bl, t = divmod(i, NT)
                ib = bl * NT
                nkc = t + 1
                # two pairs per matmul: moving = [64, 2 pairs, 128 q] (256
                # cols); A/B halves alternate PE row-tiles so ldweights hide
                qkt = qt_tiles[i]
                qtt2A = qkt[0:64, 2 * pg:2 * pg + 2, :].rearrange(
                    "p a b -> p (a b)")
                qtt2B = qkt[64:128, 2 * pg:2 * pg + 2, :].rearrange(
                    "p a b -> p (a b)")
                gk = min(2, nkc - g0)
                scA = psc.tile([128, 512], f32, tag="sc",
                               name=f"scA{i}_{pg}_{g0}")
                scB = psc.tile([128, 512], f32, tag="sc",
                               name=f"scB{i}_{pg}_{g0}")
                for gi in range(gk):
                    ktt = kt_tiles[ib + g0 + gi]
                    o = gi * 256
                    nc.tensor.matmul(scA[:, o:o + 256],
                                     ktt[0:64, :], qtt2A,
                                     start=True, stop=True)
                    nc.tensor.matmul(scB[:, o:o + 256],
                                     ktt[64:128, :], qtt2B,
                                     start=True, stop=True)
                if (i, g0) not in pt_tiles:
                    pt_tiles[(i, g0)] = (
                        ptp.tile([128, 2, 512], b16, tag="pt", bufs=20,
                                 name=f"ptA{i}_{g0}"),
                        ptp.tile([128, 2, 512], b16, tag="pt", bufs=20,
                                 name=f"ptB{i}_{g0}"))
                ptA2, ptB2 = pt_tiles[(i, g0)]
                ptA, ptB = ptA2[:, pg, :], ptB2[:, pg, :]
                nc.scalar.activation(ptA[:, 0:gk * 256], scA[:, 0:gk * 256],
                                     Exp, scale=SCALE)
                nc.scalar.activation(ptB[:, 0:gk * 256], scB[:, 0:gk * 256],
                                     Exp, scale=SCALE)
                if g0 + gk == nkc:   # diagonal chunk is last
                    o = (gk - 1) * 256
                    nc.gpsimd.tensor_mul(ptA[:, o:o + 256],
                                         ptA[:, o:o + 256],
                                         mask_sb[:, 0:256])
                    nc.gpsimd.tensor_mul(ptB[:, o:o + 256],
                                         ptB[:, o:o + 256],
                                         mask_sb[:, 0:256])

            def score_groups_of(i):
                bl, t = divmod(i, NT)
                return [(pg, g0) for pg in range(PAIRS // 2)
                        for g0 in range(0, t + 1, 2)]

            def av_alloc(i):
                return [pav.tile([128, 512], f32, tag="av", name=f"av{i}_{kv}")
                        for kv in range(KVH)]

            def av_pass(i, avps, kv):
                # transposed AV: per (kv, kc) ONE F=512 matmul, stationary
                # [V|ones]; moving = both pg groups' P^T for chunk kc via a
                # 3D strided AP [128, (pg 2, stride 512), 256]
                bl, t = divmod(i, NT)
                nkc = t + 1
                avp = avps[kv]
                for kc in range(nkc):
                    g0, gi = (kc // 2) * 2, kc % 2
                    ptA2, ptB2 = pt_tiles[(i, g0)]
                    pt_ = ptA2 if kv == 0 else ptB2
                    o = gi * 256
                    nc.tensor.matmul(avp, varena[bl][kv][:, kc, :],
                                     pt_[:, :, o:o + 256],
                                     start=(kc == 0), stop=(kc == nkc - 1))

            def av_done(i):
                bl, t = divmod(i, NT)
                for g02 in range(0, t + 1, 2):
                    pt_tiles.pop((i, g02), None)

            def av_normalize(i, avps, aT_sb, kv):
                # rows 64:128 of avp = 64 copies of the softmax denominator
                # (approx-recip is a custom DVE op: needs SBUF input)
                avp = avps[kv]
                denb = anp.tile([64, 512], f32, tag="denb",
                                name=f"db{i}_{kv}")
                nc.scalar.copy(denb, avp[64:128, :])
                recip = anp.tile([64, 512], f32, tag="recip",
                                 name=f"rc{i}_{kv}")
                nc.vector.reciprocal_approx_fast(out=recip, in_=denb)
                nc.vector.tensor_mul(
                    aT_sb[kv * 64:(kv + 1) * 64, :, :].rearrange(
                        "p a b -> p (a b)"),
                    avp[0:64, :], recip)

            def wo_tail(i, aT_sb, halves=(0, 1), out_sb=None):
                if out_sb is None:
                    out_sb = outp.tile([128, D], b16, tag="out", name=f"o{i}")
                # two 512-wide output chunks in flight so each attnT pair
                # stationary is reused for 2 consecutive matmuls
                last = (i == order[-1])
                for half in halves:
                    wo_ps = [pav.tile([128, 512], f32, tag="av",
                                      name=f"wops{i}_{half}_{n2}")
                             for n2 in range(2)]
                    for p in range(PAIRS):
                        for n2 in range(2):
                            n = half * 2 + n2
                            nc.tensor.matmul(wo_ps[n2], aT_sb[:, p, :],
                                             wo_sb[:, p, n * 512:(n + 1) * 512],
                                             start=(p == 0), stop=(p == PAIRS - 1))
                    for n2 in range(2):
                        n = half * 2 + n2
                        sl = slice(n * 512, (n + 1) * 512)
                        if (half, n2) == (1, 1) and not last:
                            nc.scalar.copy(out_sb[:, sl], wo_ps[n2])
                        else:
                            nc.vector.tensor_copy(out_sb[:, sl], wo_ps[n2])
                        if last:
                            # drain the final tile in 512-col chunks across
                            # all three DGE queues to shorten the tail
                            eng = (nc.sync, nc.gpsimd, nc.scalar, nc.sync)[n]
                            eng.dma_start(out=y_d[i][:, sl], in_=out_sb[:, sl])
                    if not last:
                        sl2 = slice(half * 1024, (half + 1) * 1024)
                        (nc.gpsimd if i % 2 else nc.sync).dma_start(
                            out=y_d[i][:, sl2], in_=out_sb[:, sl2])
                return out_sb

            # software pipeline, batches interleaved; av(i-1) matmuls are
            # issued BEFORE phase1(i) so the reciprocal+normalize (ACT/DVE)
            # chain runs under the projection matmuls, and wo(i-1) follows
            # phase1(i) with its stationaries ready
            order = []
            for t in range(NT):
                order.append(t)
                order.append(NT + t)
            for idx, i in enumerate(order):
                phase1(i, idx)
                iprev = order[idx - 1] if idx >= 1 else None
                # av(i-1) after phase1(i): the projection window lets the
                # previous tile's exp stream drain, so the av matmuls do not
                # wait; their normalize chains hide under scores(i), and
                # wo(i-1) runs last with its stationaries ready
                if iprev is not None:
                    avps = av_alloc(iprev)
                    aT_prev = anp.tile([128, PAIRS, 128], b16, tag="att",
                                       name=f"aT{iprev}")
                    for kv in range(KVH):
                        av_pass(iprev, avps, kv)
                        av_normalize(iprev, avps, aT_prev, kv)
                    av_done(iprev)
                sgs = score_groups_of(i)
                osb = None
                final = (idx == len(order) - 1)
                for gj, g in enumerate(sgs):
                    score_group(i, *g)
                    # big tiles: wo(i-1) halves slotted into the scores
                    # stream so its matmuls absorb the ACT exp drain latency
                    if iprev is not None and len(sgs) > 4:
                        if gj == 1:
                            osb = wo_tail(iprev, aT_prev, halves=(0,))
                        elif gj == 3:
                            wo_tail(iprev, aT_prev, halves=(1,), out_sb=osb)
                            osb = "done"
                    elif iprev is not None and len(sgs) > 3 and gj == 1:
                        wo_tail(iprev, aT_prev)
                        osb = "done"
                if iprev is not None and osb != "done":
                    if osb is not None:
                        wo_tail(iprev, aT_prev, halves=(1,), out_sb=osb)
                    else:
                        wo_tail(iprev, aT_prev)
            ilast = order[-1]
            avps = av_alloc(ilast)
            aT_sb = anp.tile([128, PAIRS, 128], b16, tag="att", name=f"aT{ilast}")
            for kv in range(KVH):
                av_pass(ilast, avps, kv)
                av_normalize(ilast, avps, aT_sb, kv)
            av_done(ilast)
            wo_tail(ilast, aT_sb)

    nc.compile()
    return nc


def _prep_core(x, pos_cos, pos_sin, wq, wk, wv, wo, tp, dp):
    gh = [tp * QH + h for h in PERM]
    qcols = np.concatenate([np.arange(g * HD, (g + 1) * HD) for g in gh])
    # [group, 128, 4, 512]: D-row index = g*512 + j*128 + p
    wqr = (wq[:, qcols].astype(bf)
           .reshape(DC // 4, 4, 128, QH * HD).transpose(0, 2, 1, 3).copy())
    kvc = np.arange(tp * KVH * HD, (tp + 1) * KVH * HD)
    wkv = np.concatenate([wk[:, kvc], wv[:, kvc]], axis=1)   # [D, 256]
    wkvr = (wkv.astype(bf)
            .reshape(DC // 4, 4, 128, 2 * KVH * HD).transpose(0, 2, 1, 3).copy())
    wor = (wo[qcols, :].astype(bf)
           .reshape(PAIRS, 128, D).transpose(1, 0, 2).copy())
    xs = x[dp * BL:(dp + 1) * BL]
    xt = (xs.reshape(BL, NT, 128, DC, 128).transpose(0, 1, 4, 3, 2)
          .reshape(NQT, 128, DC, 128).astype(bf))
    cosr = (np.tile(pos_cos, (1, QH)).astype(np.float32)
            .reshape(NT, 128, QH * HD // 2).transpose(1, 0, 2).astype(bf).copy())
    sinr = (np.tile(pos_sin, (1, QH)).astype(np.float32)
            .reshape(NT, 128, QH * HD // 2).transpose(1, 0, 2).astype(bf).copy())
    maskr = np.tile(np.triu(np.ones((128, 128), np.float32)), (1, 4)).astype(bf)
    identr = np.eye(128, dtype=np.float32).astype(bf)
    return {"xt": np.ascontiguousarray(xt), "wqr": wqr, "wkvr": wkvr,
            "wor": wor, "cosr": cosr, "sinr": sinr,
            "maskr": maskr, "identr": identr}


def make_in_maps(x, pos_cos, pos_sin, wq, wk, wv, wo):
    x = np.asarray(x, np.float32)
    pos_cos = np.asarray(pos_cos, np.float32)
    pos_sin = np.asarray(pos_sin, np.float32)
    wq = np.asarray(wq, np.float32)
    wk = np.asarray(wk, np.float32)
    wv = np.asarray(wv, np.float32)
    wo = np.asarray(wo, np.float32)
    return [_prep_core(x, pos_cos, pos_sin, wq, wk, wv, wo, c % TP, c // TP)
            for c in range(TP * DP)]


def gather(results):
    y = np.empty((B, S, D), np.float32)
    for dp in range(DP):
        acc = results[dp * TP]["y"].astype(np.float32).copy()
        for t in range(1, TP):
            acc += results[dp * TP + t]["y"]
        y[dp * BL:(dp + 1) * BL] = acc.reshape(BL, S, D)
    return y


def get_nc():
    global _built
    if _built is None:
        _built = _build()
    return _built


def kernel(x, pos_cos, pos_sin, wq, wk, wv, wo):
    from concourse.bass_utils import run_bass_kernel_spmd
    nc = get_nc()
    in_maps = make_in_maps(x, pos_cos, pos_sin, wq, wk, wv, wo)
    res = run_bass_kernel_spmd(nc, in_maps, list(range(TP * DP)))
    return gather(res.results)
